# revision 15
# baseline (speedup 1.0000x reference)
"""Trainium2 Bass kernel for nn_DivergenceRN (gnn_message_passing).

Reference computes, per batch b:
    Z_XX[b,i,:] = max_j relu(X[b,j]@W1a_xx + X[b,i]@W1c_xx + b1_xx) @ W_xx2
    Z_YX[b,i,:] = max_j relu(Y[b,j]@W1a_yx + X[b,i]@W1c_yx + b1_yx) @ W_yx2
    Z = sum_i (Z_XX - Z_YX);  out = relu(cat(Z,Z)@Wd1+bd1)@Wd2+bd2
(The YY / XY branches in the reference are dead code — output-independent.)

Structure (v3), driven by measured TRN2 engine rates:
  * The j-part PA[b] = W1ad^T @ [X^T;Y^T][b] is identical for every i —
    computed once per b and cached in SBUF as bf16 (the baseline re-did
    this matmul for all 192 pairs).
  * Per (b,i) pair: rp = relu(PA[b] + PC[b][:,i]) then h = W2bd^T @ rp
    then strip[:,b,i] = max_j h.  Engine assignment per measurements:
      - relu: scalar-engine activation with per-partition bias ptr
        (~610ns), except every 7th i runs on DVE as u = max(PA, -PC_i)
        via tensor_tensor with a broadcast [P,1] operand (~555ns).  The
        dropped "+PC_i" constant commutes out of the max and is restored
        by appending q_i = W2bd^T @ PC_i columns to the strip before the
        final i-sum (one tiny extra matmul for all such i).
      - W2 matmuls are packed 512-wide across pair boundaries: a 4-pair
        group is 3 matmuls into 3 PSUM banks (fewer LDWEIGHTS, fuller PE).
      - max: direct DVE tensor_reduce over the packed [4,384] PSUM view
        (~1750ns per group).  Measured: no dtype/space variant is faster,
        and TensorScalarPtr/GpSimd paths are 5-15x slower than the
        cost model claims — avoided entirely.
Sharding: i in [0,384) split across 8 cores (48 rows per core per batch).
Host does the final cross-core sum + b2/decoder folding (tiny).
"""

import numpy as np

import concourse.bacc as bacc
import concourse.mybir as mybir
import concourse.tile as tile
from concourse.bass_utils import run_bass_kernel_spmd

B, N, M, D, H = 4, 384, 384, 64, 64
NCORES = 8
NI = N // NCORES          # i-rows per core per batch
P = 2 * H                 # 128 partitions: h x {xx, yx}
BLOB_W = B * N + B * NI + 2 * P   # packed input blob columns
G = 4                     # pairs per group (3 x 512-col matmuls)
USTRIDE = 12              # every USTRIDE-th i uses the DVE u-path
NU = (NI + USTRIDE - 1) // USTRIDE   # u-pairs per (core, b)

F32 = mybir.dt.float32
F32R = mybir.dt.float32r
BF16 = mybir.dt.bfloat16
AX = mybir.AxisListType
ALU = mybir.AluOpType
ACTF = mybir.ActivationFunctionType


def build_nc():
    nc = bacc.Bacc("TRN2", target_bir_lowering=False)

    # All f32r inputs packed into one [128, BLOB_W] blob: a single DMA =
    # a single semaphore (per-instruction sync-wait limits are tiny).
    blob = nc.dram_tensor("blob", [P, BLOB_W], F32R, kind="ExternalInput")
    w2bd16 = nc.dram_tensor("w2bd16", [P, P], BF16, kind="ExternalInput")
    out = nc.dram_tensor("out", [P, B], F32, kind="ExternalOutput")

    with tile.TileContext(nc) as tc:
        with (
            tc.tile_pool(name="singles", bufs=1) as singles,
            tc.tile_pool(name="rp", bufs=6) as rp_pool,
            tc.tile_pool(name="pap", bufs=1, space="PSUM") as pa_pool,
            tc.tile_pool(name="hps", bufs=2, space="PSUM") as h_pool,
        ):
            blob_s = singles.tile([P, BLOB_W], F32R)
            w2bd_s16 = singles.tile([P, P], BF16)
            pcf = singles.tile([P, B, NI], F32)
            pc16 = singles.tile([P, B, NI], BF16)
            npc16 = singles.tile([P, B, NI], BF16)
            strip = singles.tile([P, B, NI + NU], F32)
            acc = singles.tile([P, B], F32)

            # Warm the Relu activation table before the main loop.
            warm = singles.tile([P, 8], F32)
            nc.vector.memset(warm, 0.0)
            nc.scalar.activation(
                out=warm[:, 0:1], in_=warm[:, 0:1], func=ACTF.Relu, scale=1.0
            )
            # Blob layout [xitb|w1ad|w1cb|XYT]: one merged DMA covers the
            # weights block plus XYT[b=0], so PC/PA0 matmuls start as early
            # as possible; remaining XYT chunks ride the idle gpsimd queue.
            W0 = B * NI + 2 * P
            nc.sync.dma_start(out=blob_s[:, 0:W0], in_=blob[:, 0:W0])
            nc.sync.dma_start(
                out=blob_s[:, W0 : W0 + N], in_=blob[:, W0 : W0 + N]
            )
            nc.sync.dma_start(out=w2bd_s16, in_=w2bd16[:, :])
            for b in range(1, B):
                nc.gpsimd.dma_start(
                    out=blob_s[:, W0 + b * N : W0 + (b + 1) * N],
                    in_=blob[:, W0 + b * N : W0 + (b + 1) * N],
                )
            o = 0
            # xitb: [65, B, NI] — rows 0-63 Xi^T, row 64 = 1.0 (bias lane)
            xitb_s = blob_s[0 : D + 1, o : o + B * NI]
            o += B * NI
            w1ad_s = blob_s[:, o : o + P]
            o += P
            # w1cb: [65, 128] — rows 0-63 = [W1c_xx | W1c_yx], row 64 = b1^T
            w1cb_s = blob_s[0 : D + 1, o : o + P]
            o += P
            xyt_s = blob_s[:, o : o + B * N].rearrange("p (b n) -> p b n", b=B)
            o += B * N
            assert o == BLOB_W

            # Phase 0: PA[b] = W1ad^T @ XYT[b] (f32r, once per b) -> bf16
            # SBUF; PC (all b, one matmul) -> f32 + (+/-)bf16 copies.
            pc_ps = pa_pool.tile([P, 2, 512], F32, tag="pa")
            nc.tensor.matmul(
                pc_ps[:, 0, 0 : B * NI],
                lhsT=w1cb_s,
                rhs=xitb_s,
                start=True,
                stop=True,
            )
            nc.scalar.activation(
                out=pcf.rearrange("p b i -> p (b i)"),
                in_=pc_ps[:, 0, 0 : B * NI],
                func=ACTF.Identity,
                scale=1.0,
            )
            nc.scalar.activation(
                out=pc16.rearrange("p b i -> p (b i)"),
                in_=pc_ps[:, 0, 0 : B * NI],
                func=ACTF.Identity,
                scale=1.0,
            )
            nc.scalar.activation(
                out=npc16.rearrange("p b i -> p (b i)"),
                in_=pc_ps[:, 0, 0 : B * NI],
                func=ACTF.Identity,
                scale=-1.0,
            )
            # Main loop: PA[b] stays resident in a 2-bank PSUM tile per
            # half-batch (no SBUF copy); relu/u ops read it from PSUM.
            for half in range(2):
                pa_ps = pa_pool.tile([P, 2, 512], F32, tag="pa")
                for k in range(2):
                    b = half * 2 + k
                    nc.tensor.matmul(
                        pa_ps[:, k, 0:N],
                        lhsT=w1ad_s,
                        rhs=xyt_s[:, b, :],
                        start=True,
                        stop=True,
                    )
                for k in range(2):
                    b = half * 2 + k
                    for ig in range(NI // G):
                        i0 = ig * G
                        rp = rp_pool.tile([P, G, N], BF16, tag="rp")
                        for g in range(G):
                            i = i0 + g
                            if i % USTRIDE == 0:
                                nc.vector.tensor_tensor(
                                    out=rp[:, g, :],
                                    in0=pa_ps[:, k, 0:N],
                                    in1=npc16[:, b, i : i + 1].broadcast_to(
                                        [P, N]
                                    ),
                                    op=ALU.max,
                                )
                            else:
                                nc.scalar.activation(
                                    out=rp[:, g, :],
                                    in_=pa_ps[:, k, 0:N],
                                    func=ACTF.Relu,
                                    bias=pcf[:, b, i : i + 1],
                                    scale=1.0,
                                )
                        h_ps = h_pool.tile([P, 3, 512], F32, tag="h")
                        hflat = h_ps.rearrange("p a c -> p (a c)")
                        rpflat = rp.rearrange("p g n -> p (g n)")
                        for m in range(3):
                            nc.tensor.matmul(
                                hflat[:, 512 * m : 512 * (m + 1)],
                                lhsT=w2bd_s16,
                                rhs=rpflat[:, 512 * m : 512 * (m + 1)],
                                start=True,
                                stop=True,
                            )
                        nc.vector.tensor_reduce(
                            out=strip[:, b, i0 : i0 + G],
                            in_=hflat.rearrange("p (g n) -> p g n", g=G),
                            axis=AX.X,
                            op=ALU.max,
                        )

            # q columns: q[:, b, u] = W2bd^T @ PC16[:, b, u*USTRIDE], one
            # matmul for all (b, u); appended to strip so the final i-sum
            # restores the "+PC_i" constant dropped by the DVE u-path.
            q_ps = pa_pool.tile([P, 2, 512], F32, tag="pa")
            nc.tensor.matmul(
                q_ps[:, 0, 0 : B * NU],
                lhsT=w2bd_s16,
                rhs=pc16[:, :, 0:NI:USTRIDE],
                start=True,
                stop=True,
            )
            nc.vector.tensor_scalar(
                out=strip[:, :, NI : NI + NU],
                in0=q_ps[:, 0, 0 : B * NU].rearrange("p (b u) -> p b u", b=B),
                scalar1=0.0,
                scalar2=None,
                op0=ALU.add,
            )
            nc.vector.tensor_reduce(
                out=acc[:, :], in_=strip[:, :, :], axis=AX.X, op=ALU.add
            )
            nc.sync.dma_start(out=out[:, :], in_=acc[:, :])

    nc.compile()
    return nc


def _prep_inputs(X, Y, W_xx1, W_yx1, b_xx1, b_yx1, W_xx2, W_yx2):
    """Host-side input prep shared by all cores (except xit)."""
    f = np.float32
    XYT = np.ascontiguousarray(
        np.concatenate([X.transpose(0, 2, 1), Y.transpose(0, 2, 1)], axis=1), f
    )  # [B, 128, N]
    W1ad = np.zeros((P, P), f)
    W1ad[:D, :H] = W_xx1[:D]
    W1ad[D:, H:] = W_yx1[:D]
    W1c = np.ascontiguousarray(np.concatenate([W_xx1[D:], W_yx1[D:]], axis=1), f)
    b1v = np.concatenate([b_xx1, b_yx1]).reshape(P, 1).astype(f)
    W2bd = np.zeros((P, P), f)
    W2bd[:H, :H] = W_xx2
    W2bd[H:, H:] = W_yx2
    return XYT, W1ad, W1c, b1v, W2bd


def _pack_blob(XYT, XiT, W1ad, W1c, b1v, W2bd):
    """Pack per-core inputs into the [P, BLOB_W] blob (see build_nc)."""
    f = np.float32
    blob = np.zeros((P, BLOB_W), f)
    o = 0
    blob[:D, o : o + B * NI] = XiT.transpose(1, 0, 2).reshape(D, B * NI)
    blob[D, o : o + B * NI] = 1.0
    o += B * NI
    blob[:, o : o + P] = W1ad
    o += P
    blob[:D, o : o + P] = W1c
    blob[D, o : o + P] = b1v[:, 0]
    o += P
    blob[:, o : o + B * N] = XYT.transpose(1, 0, 2).reshape(P, B * N)
    o += B * N
    assert o == BLOB_W
    return blob


def kernel(
    X, Y,
    W_xx1, b_xx1, W_xx2, b_xx2,
    W_xy1, b_xy1, W_xy2, b_xy2,
    W_yx1, b_yx1, W_yx2, b_yx2,
    W_yy1, b_yy1, W_yy2, b_yy2,
    Wd1, bd1, Wd2, bd2,
    _trace=False, _tmpdir=None,
):
    f = np.float32
    X = np.asarray(X, f)
    Y = np.asarray(Y, f)
    XYT, W1ad, W1c, b1v, W2bd = _prep_inputs(
        X, Y, W_xx1, W_yx1, b_xx1, b_yx1, W_xx2, W_yx2
    )
    import ml_dtypes
    W2bd16 = np.ascontiguousarray(W2bd.astype(ml_dtypes.bfloat16))

    in_maps = []
    for c in range(NCORES):
        XiT = np.ascontiguousarray(
            X[:, c * NI : (c + 1) * NI, :].transpose(0, 2, 1), f
        )  # [B, 64, NI]
        in_maps.append(
            {
                "blob": _pack_blob(XYT, XiT, W1ad, W1c, b1v, W2bd),
                "w2bd16": W2bd16,
            }
        )

    nc = build_nc()
    res = run_bass_kernel_spmd(
        nc,
        in_maps,
        core_ids=list(range(NCORES)),
        trace=_trace,
        tmpdir=_tmpdir,
    )
    acc = np.zeros((P, B), np.float64)
    for r in res.results:
        acc += r["out"].astype(np.float64)
    acc = acc.astype(f)

    # acc[k, b] = sum_i max_j (relu_pre @ W2)[k]  for xx (k<64) / yx (k>=64)
    Zdiff = (acc[:H] - acc[H:]).T + N * (b_xx2 - b_yx2)[None, :]  # [B, H]
    z = np.concatenate([Zdiff, Zdiff], axis=1).astype(f)  # [B, 2H]
    h = np.maximum(z @ Wd1 + bd1, 0.0).astype(f)
    outv = (h @ Wd2 + bd2).astype(f)
    if _trace:
        return outv, res
    return outv


# revision 16
# speedup vs baseline: 1.2209x; 1.2209x over previous
"""Trainium2 Bass kernel for nn_DivergenceRN (gnn_message_passing).

Reference computes, per batch b:
    Z_XX[b,i,:] = max_j relu(X[b,j]@W1a_xx + X[b,i]@W1c_xx + b1_xx) @ W_xx2
    Z_YX[b,i,:] = max_j relu(Y[b,j]@W1a_yx + X[b,i]@W1c_yx + b1_yx) @ W_yx2
    Z = sum_i (Z_XX - Z_YX);  out = relu(cat(Z,Z)@Wd1+bd1)@Wd2+bd2
(The YY / XY branches in the reference are dead code — output-independent.)

Structure (v3), driven by measured TRN2 engine rates:
  * The j-part PA[b] = W1ad^T @ [X^T;Y^T][b] is identical for every i —
    computed once per b and cached in SBUF as bf16 (the baseline re-did
    this matmul for all 192 pairs).
  * Per (b,i) pair: rp = relu(PA[b] + PC[b][:,i]) then h = W2bd^T @ rp
    then strip[:,b,i] = max_j h.  Engine assignment per measurements:
      - relu: scalar-engine activation with per-partition bias ptr
        (~610ns), except every 7th i runs on DVE as u = max(PA, -PC_i)
        via tensor_tensor with a broadcast [P,1] operand (~555ns).  The
        dropped "+PC_i" constant commutes out of the max and is restored
        by appending q_i = W2bd^T @ PC_i columns to the strip before the
        final i-sum (one tiny extra matmul for all such i).
      - W2 matmuls are packed 512-wide across pair boundaries: a 4-pair
        group is 3 matmuls into 3 PSUM banks (fewer LDWEIGHTS, fuller PE).
      - max: direct DVE tensor_reduce over the packed [4,384] PSUM view
        (~1750ns per group).  Measured: no dtype/space variant is faster,
        and TensorScalarPtr/GpSimd paths are 5-15x slower than the
        cost model claims — avoided entirely.
Sharding: i in [0,384) split across 8 cores (48 rows per core per batch).
Host does the final cross-core sum + b2/decoder folding (tiny).
"""

import numpy as np

import concourse.bacc as bacc
import concourse.mybir as mybir
import concourse.tile as tile
from concourse.bass_utils import run_bass_kernel_spmd

B, N, M, D, H = 4, 384, 384, 64, 64
NCORES = 8
NI = N // NCORES          # i-rows per core per batch
P = 2 * H                 # 128 partitions: h x {xx, yx}
BLOB_W = B * N + B * NI + 2 * P   # packed input blob columns
G = 4                     # pairs per group (3 x 512-col matmuls)
USTRIDE = 12              # every USTRIDE-th i uses the DVE u-path
NU = (NI + USTRIDE - 1) // USTRIDE   # u-pairs per (core, b)

F32 = mybir.dt.float32
F32R = mybir.dt.float32r
BF16 = mybir.dt.bfloat16
AX = mybir.AxisListType
ALU = mybir.AluOpType
ACTF = mybir.ActivationFunctionType


def build_nc():
    nc = bacc.Bacc("TRN2", target_bir_lowering=False)

    # All f32r inputs packed into one [128, BLOB_W] blob: a single DMA =
    # a single semaphore (per-instruction sync-wait limits are tiny).
    blob = nc.dram_tensor("blob", [P, BLOB_W], F32R, kind="ExternalInput")
    w2bd16 = nc.dram_tensor("w2bd16", [P, P], BF16, kind="ExternalInput")
    out = nc.dram_tensor("out", [P, B], F32, kind="ExternalOutput")

    with tile.TileContext(nc) as tc:
        with (
            tc.tile_pool(name="singles", bufs=1) as singles,
            tc.tile_pool(name="rp", bufs=6) as rp_pool,
            tc.tile_pool(name="pap", bufs=1, space="PSUM") as pa_pool,
            tc.tile_pool(name="hps", bufs=2, space="PSUM") as h_pool,
        ):
            blob_s = singles.tile([P, BLOB_W], F32R)
            w2bd_s16 = singles.tile([P, P], BF16)
            pa16 = singles.tile([P, B, N], BF16)
            pcf = singles.tile([P, B, NI], F32)
            pc16 = singles.tile([P, B, NI], BF16)
            npc16 = singles.tile([P, B, NI], BF16)
            strip = singles.tile([P, B, NI + NU], F32)
            acc = singles.tile([P, B], F32)

            # Warm the Relu activation table before the main loop.
            warm = singles.tile([P, 8], F32)
            nc.vector.memset(warm, 0.0)
            nc.scalar.activation(
                out=warm[:, 0:1], in_=warm[:, 0:1], func=ACTF.Relu, scale=1.0
            )
            # Blob layout [xitb|w1ad|w1cb|XYT]: one merged DMA covers the
            # weights block plus XYT[b=0], so PC/PA0 matmuls start as early
            # as possible; remaining XYT chunks ride the idle gpsimd queue.
            W0 = B * NI + 2 * P
            nc.sync.dma_start(out=blob_s[:, 0:W0], in_=blob[:, 0:W0])
            nc.sync.dma_start(
                out=blob_s[:, W0 : W0 + N], in_=blob[:, W0 : W0 + N]
            )
            nc.sync.dma_start(out=w2bd_s16, in_=w2bd16[:, :])
            for b in range(1, B):
                nc.gpsimd.dma_start(
                    out=blob_s[:, W0 + b * N : W0 + (b + 1) * N],
                    in_=blob[:, W0 + b * N : W0 + (b + 1) * N],
                )
            o = 0
            # xitb: [65, B, NI] — rows 0-63 Xi^T, row 64 = 1.0 (bias lane)
            xitb_s = blob_s[0 : D + 1, o : o + B * NI]
            o += B * NI
            w1ad_s = blob_s[:, o : o + P]
            o += P
            # w1cb: [65, 128] — rows 0-63 = [W1c_xx | W1c_yx], row 64 = b1^T
            w1cb_s = blob_s[0 : D + 1, o : o + P]
            o += P
            xyt_s = blob_s[:, o : o + B * N].rearrange("p (b n) -> p b n", b=B)
            o += B * N
            assert o == BLOB_W

            # Phase 0: PA[b] = W1ad^T @ XYT[b] (f32r, once per b) -> bf16
            # SBUF; PC (all b, one matmul) -> f32 + (+/-)bf16 copies.
            pc_ps = pa_pool.tile([P, 2, 512], F32, tag="pa")
            nc.tensor.matmul(
                pc_ps[:, 0, 0 : B * NI],
                lhsT=w1cb_s,
                rhs=xitb_s,
                start=True,
                stop=True,
            )
            nc.scalar.activation(
                out=pcf.rearrange("p b i -> p (b i)"),
                in_=pc_ps[:, 0, 0 : B * NI],
                func=ACTF.Identity,
                scale=1.0,
            )
            nc.scalar.activation(
                out=pc16.rearrange("p b i -> p (b i)"),
                in_=pc_ps[:, 0, 0 : B * NI],
                func=ACTF.Identity,
                scale=1.0,
            )
            nc.scalar.activation(
                out=npc16.rearrange("p b i -> p (b i)"),
                in_=pc_ps[:, 0, 0 : B * NI],
                func=ACTF.Identity,
                scale=-1.0,
            )
            for half in range(2):
                pa_ps = pa_pool.tile([P, 2, 512], F32, tag="pa")
                for k in range(2):
                    b = half * 2 + k
                    nc.tensor.matmul(
                        pa_ps[:, k, 0:N],
                        lhsT=w1ad_s,
                        rhs=xyt_s[:, b, :],
                        start=True,
                        stop=True,
                    )
                    if half == 0:
                        nc.vector.tensor_scalar(
                            out=pa16[:, b : b + 1, :],
                            in0=pa_ps[:, k : k + 1, 0:N],
                            scalar1=0.0,
                            scalar2=None,
                            op0=ALU.add,
                        )
                if half == 1:
                    nc.vector.tensor_scalar(
                        out=pa16[:, 2:4, :],
                        in0=pa_ps[:, :, 0:N],
                        scalar1=0.0,
                        scalar2=None,
                        op0=ALU.add,
                    )

            # Main loop: per group of G=4 pairs.
            for b in range(B):
                for ig in range(NI // G):
                    i0 = ig * G
                    rp = rp_pool.tile([P, G, N], BF16, tag="rp")
                    for g in range(G):
                        i = i0 + g
                        if i % USTRIDE == 0:
                            nc.vector.tensor_tensor(
                                out=rp[:, g, :],
                                in0=pa16[:, b, :],
                                in1=npc16[:, b, i : i + 1].broadcast_to(
                                    [P, N]
                                ),
                                op=ALU.max,
                            )
                        else:
                            nc.scalar.activation(
                                out=rp[:, g, :],
                                in_=pa16[:, b, :],
                                func=ACTF.Relu,
                                bias=pcf[:, b, i : i + 1],
                                scale=1.0,
                            )
                    h_ps = h_pool.tile([P, 3, 512], F32, tag="h")
                    hflat = h_ps.rearrange("p a c -> p (a c)")
                    rpflat = rp.rearrange("p g n -> p (g n)")
                    for k in range(3):
                        nc.tensor.matmul(
                            hflat[:, 512 * k : 512 * (k + 1)],
                            lhsT=w2bd_s16,
                            rhs=rpflat[:, 512 * k : 512 * (k + 1)],
                            start=True,
                            stop=True,
                        )
                    nc.vector.tensor_reduce(
                        out=strip[:, b, i0 : i0 + G],
                        in_=hflat.rearrange("p (g n) -> p g n", g=G),
                        axis=AX.X,
                        op=ALU.max,
                    )

            # q columns: q[:, b, u] = W2bd^T @ PC16[:, b, u*USTRIDE], one
            # matmul for all (b, u); appended to strip so the final i-sum
            # restores the "+PC_i" constant dropped by the DVE u-path.
            q_ps = pa_pool.tile([P, 2, 512], F32, tag="pa")
            nc.tensor.matmul(
                q_ps[:, 0, 0 : B * NU],
                lhsT=w2bd_s16,
                rhs=pc16[:, :, 0:NI:USTRIDE],
                start=True,
                stop=True,
            )
            nc.vector.tensor_scalar(
                out=strip[:, :, NI : NI + NU],
                in0=q_ps[:, 0, 0 : B * NU].rearrange("p (b u) -> p b u", b=B),
                scalar1=0.0,
                scalar2=None,
                op0=ALU.add,
            )
            nc.vector.tensor_reduce(
                out=acc[:, :], in_=strip[:, :, :], axis=AX.X, op=ALU.add
            )
            nc.sync.dma_start(out=out[:, :], in_=acc[:, :])

    nc.compile()
    return nc


def _prep_inputs(X, Y, W_xx1, W_yx1, b_xx1, b_yx1, W_xx2, W_yx2):
    """Host-side input prep shared by all cores (except xit)."""
    f = np.float32
    XYT = np.ascontiguousarray(
        np.concatenate([X.transpose(0, 2, 1), Y.transpose(0, 2, 1)], axis=1), f
    )  # [B, 128, N]
    W1ad = np.zeros((P, P), f)
    W1ad[:D, :H] = W_xx1[:D]
    W1ad[D:, H:] = W_yx1[:D]
    W1c = np.ascontiguousarray(np.concatenate([W_xx1[D:], W_yx1[D:]], axis=1), f)
    b1v = np.concatenate([b_xx1, b_yx1]).reshape(P, 1).astype(f)
    W2bd = np.zeros((P, P), f)
    W2bd[:H, :H] = W_xx2
    W2bd[H:, H:] = W_yx2
    return XYT, W1ad, W1c, b1v, W2bd


def _pack_blob(XYT, XiT, W1ad, W1c, b1v, W2bd):
    """Pack per-core inputs into the [P, BLOB_W] blob (see build_nc)."""
    f = np.float32
    blob = np.zeros((P, BLOB_W), f)
    o = 0
    blob[:D, o : o + B * NI] = XiT.transpose(1, 0, 2).reshape(D, B * NI)
    blob[D, o : o + B * NI] = 1.0
    o += B * NI
    blob[:, o : o + P] = W1ad
    o += P
    blob[:D, o : o + P] = W1c
    blob[D, o : o + P] = b1v[:, 0]
    o += P
    blob[:, o : o + B * N] = XYT.transpose(1, 0, 2).reshape(P, B * N)
    o += B * N
    assert o == BLOB_W
    return blob


def kernel(
    X, Y,
    W_xx1, b_xx1, W_xx2, b_xx2,
    W_xy1, b_xy1, W_xy2, b_xy2,
    W_yx1, b_yx1, W_yx2, b_yx2,
    W_yy1, b_yy1, W_yy2, b_yy2,
    Wd1, bd1, Wd2, bd2,
    _trace=False, _tmpdir=None,
):
    f = np.float32
    X = np.asarray(X, f)
    Y = np.asarray(Y, f)
    XYT, W1ad, W1c, b1v, W2bd = _prep_inputs(
        X, Y, W_xx1, W_yx1, b_xx1, b_yx1, W_xx2, W_yx2
    )
    import ml_dtypes
    W2bd16 = np.ascontiguousarray(W2bd.astype(ml_dtypes.bfloat16))

    in_maps = []
    for c in range(NCORES):
        XiT = np.ascontiguousarray(
            X[:, c * NI : (c + 1) * NI, :].transpose(0, 2, 1), f
        )  # [B, 64, NI]
        in_maps.append(
            {
                "blob": _pack_blob(XYT, XiT, W1ad, W1c, b1v, W2bd),
                "w2bd16": W2bd16,
            }
        )

    nc = build_nc()
    res = run_bass_kernel_spmd(
        nc,
        in_maps,
        core_ids=list(range(NCORES)),
        trace=_trace,
        tmpdir=_tmpdir,
    )
    acc = np.zeros((P, B), np.float64)
    for r in res.results:
        acc += r["out"].astype(np.float64)
    acc = acc.astype(f)

    # acc[k, b] = sum_i max_j (relu_pre @ W2)[k]  for xx (k<64) / yx (k>=64)
    Zdiff = (acc[:H] - acc[H:]).T + N * (b_xx2 - b_yx2)[None, :]  # [B, H]
    z = np.concatenate([Zdiff, Zdiff], axis=1).astype(f)  # [B, 2H]
    h = np.maximum(z @ Wd1 + bd1, 0.0).astype(f)
    outv = (h @ Wd2 + bd2).astype(f)
    if _trace:
        return outv, res
    return outv
